# revision 1
# baseline (speedup 1.0000x reference)
"""Causal self-attention on 8 NeuronCores (Trainium2, Bass/Tile).

Problem: B=4, T=2048, C=1024, H=16 heads, HD=64, fp32.
    qkv = x @ Wqkv + bqkv ; causal softmax attention ; y @ Wproj + bproj

Sharding (Megatron-style): 8 cores = 4 batches x 2 head-groups.
Core c handles batch b = c//2 and head group g = c%2 (8 heads each).
Each core computes a partial output projection over its 512 head-dims;
the host sums the two partials per batch and adds bproj (the unshard step).

Per-core kernel (all matmuls fp32r = full PE rate, TF32-class precision):
  phase 1: qkv^T projection. Q^T,K^T produced feature-major [feat, tok]
           (lhsT = Wqk chunk, rhs = x^T chunk); V produced token-major
           [tok, feat] with a ones column appended per head (65-stride) so
           the A.V matmul also produces the softmax denominator; V bias
           folded in via a K=1 ones-row matmul; Q/K bias via DVE add.
  phase 2: per (query chunk qc of 512, head pair hp): S^T[k,q] tiles from
           row-packed K=64 matmul pairs (2 heads share the PE array via
           tile_position row groups); causal mask added on PSUM; one exp
           per pair on ACT (scale=1/8 folded in); A.V consumes P^T
           directly (lhsT = V block with ones col -> [65,q] psum: rows
           0-63 unnormalized y^T, row 64 sumexp). Softmax division is
           deferred: recip(sumexp) is broadcast across partitions with a
           K=1 outer-product matmul and applied by one DVE mul per head.
           Head B's result moves to partitions 64-127 via sbuf-sbuf DMA.
  phase 3: output projection partial (lhsT = y^T chunk, rhs = Wproj rows
           of this head group) accumulated over 4 cin chunks in PSUM.

No max-subtraction in softmax (scores are ~N(0,1) by construction, exp is
fp32-safe), no transposes, no collectives.
"""
import sys

for _p in ("/opt/trn_rl_repo",):
    if _p not in sys.path:
        sys.path.append(_p)

import numpy as np

B, T, C = 4, 2048, 1024
H, HD = 16, 64
N_CORES = 8
G_HEADS = 8            # heads per core (one group)
G_FEAT = G_HEADS * HD  # 512 feature dims per group
VW = HD + 1            # V block stride per head (64 values + ones col)

TOKC = 256             # phase-1 token chunk (fp32r needs moving dim >= 256)
QC = 512               # phase-2 query chunk
NEG = -1e30

_CACHE = {}


def _build_program():
    import contextlib
    import concourse.tile as tile
    from concourse import bacc, mybir

    F32 = mybir.dt.float32
    R32 = mybir.dt.float32r
    Exp = mybir.ActivationFunctionType.Exp

    nc = bacc.Bacc("TRN2", target_bir_lowering=False, debug=False,
                   num_devices=N_CORES)

    xT_d = nc.dram_tensor("xT", [C, T], R32, kind="ExternalInput").ap()
    wqk_d = nc.dram_tensor("wqk", [C, 2 * G_FEAT], R32, kind="ExternalInput").ap()
    wv_d = nc.dram_tensor("wv", [C, G_FEAT], R32, kind="ExternalInput").ap()
    bqk_d = nc.dram_tensor("bqk", [2 * G_FEAT], F32, kind="ExternalInput").ap()
    bv_d = nc.dram_tensor("bv", [1, G_FEAT], R32, kind="ExternalInput").ap()
    wp_d = nc.dram_tensor("wp", [G_FEAT, C], R32, kind="ExternalInput").ap()
    part_d = nc.dram_tensor("part", [T, C], F32, kind="ExternalOutput").ap()

    n_tc = T // TOKC             # 8 phase-1 token chunks
    n_cc = C // 128              # 8 contraction chunks
    n_qc = T // QC               # 4 query chunks
    n_hp = G_HEADS // 2          # 4 head pairs
    n_tb = T // 128              # 16 token blocks

    with tile.TileContext(nc) as tc, contextlib.ExitStack() as ctx:
        const = ctx.enter_context(tc.tile_pool(name="const", bufs=1))
        wpool = ctx.enter_context(tc.tile_pool(name="weights", bufs=1))
        big = ctx.enter_context(tc.tile_pool(name="big", bufs=1))
        xpool = ctx.enter_context(tc.tile_pool(name="xT", bufs=3))
        ytpool = ctx.enter_context(tc.tile_pool(name="yT", bufs=2))
        ptpool = ctx.enter_context(tc.tile_pool(name="pt", bufs=2))
        rcpool = ctx.enter_context(tc.tile_pool(name="recip", bufs=2))
        ps_acc = ctx.enter_context(
            tc.tile_pool(name="ps_acc", bufs=2, space="PSUM"))
        ps_u = ctx.enter_context(
            tc.tile_pool(name="ps_u", bufs=2, space="PSUM"))
        ps_s = ctx.enter_context(
            tc.tile_pool(name="ps_s", bufs=2, space="PSUM"))

        # ---- constants ----
        ones_f32 = const.tile([128, 128], F32)
        nc.vector.memset(ones_f32[:], 1.0)
        ones_row = const.tile([1, 128], R32)   # K=1 matmul lhsT rows
        nc.vector.tensor_copy(ones_row[:], ones_f32[0:1, :])
        # causal triangle: 0 where col >= row, NEG where col < row
        mask_tri = const.tile([128, 128], F32)
        nc.vector.memset(mask_tri[:], 0.0)
        nc.gpsimd.affine_select(
            out=mask_tri[:], in_=mask_tri[:],
            compare_op=mybir.AluOpType.is_ge, fill=NEG, base=0,
            pattern=[[1, 128]], channel_multiplier=-1)

        # ---- resident weights ----
        # wqk_sb[:, cc*1024 + f*128 : +128] = Wqk[cc*128:+128, f*128:+128]
        wqk_sb = wpool.tile([128, n_cc * 2 * G_FEAT], R32)
        for cc in range(n_cc):
            nc.sync.dma_start(
                wqk_sb[:, cc * 2 * G_FEAT:(cc + 1) * 2 * G_FEAT],
                wqk_d[cc * 128:(cc + 1) * 128, :])
        wv_sb = wpool.tile([128, n_cc * G_FEAT], R32)
        for cc in range(n_cc):
            nc.sync.dma_start(
                wv_sb[:, cc * G_FEAT:(cc + 1) * G_FEAT],
                wv_d[cc * 128:(cc + 1) * 128, :])
        wp_sb = wpool.tile([128, 4 * C], R32)
        for cc in range(4):
            nc.sync.dma_start(
                wp_sb[:, cc * C:(cc + 1) * C],
                wp_d[cc * 128:(cc + 1) * 128, :])
        bqk_sb = wpool.tile([128, 8], F32)
        nc.sync.dma_start(bqk_sb[:], bqk_d.rearrange("(f p) -> p f", p=128))
        bv_sb = wpool.tile([1, G_FEAT], R32)
        nc.sync.dma_start(bv_sb[:], bv_d[:])

        # ---- big activations ----
        qt_sb = big.tile([128, n_hp * T], R32)  # [feat, tok] head-pair major
        kt_sb = big.tile([128, n_hp * T], R32)
        # V: [tok-block, head, 64 vals + ones col]
        v_sb = big.tile([128, n_tb * G_HEADS * VW], R32)
        nc.vector.tensor_copy(
            v_sb[:].rearrange("p (t w) -> p t w", w=VW)[:, :, HD:HD + 1],
            ones_f32[:].rearrange("p (a b) -> p a b", b=1))

        # ================= phase 1: qkv projection =================
        half = n_cc // 2
        for tci in range(n_tc):
            xts = []
            for hf in range(2):
                xt = xpool.tile([128, half * TOKC], R32, tag="xT")
                for cc in range(half):
                    ccg = hf * half + cc
                    nc.sync.dma_start(
                        xt[:, cc * TOKC:(cc + 1) * TOKC],
                        xT_d[ccg * 128:(ccg + 1) * 128,
                             tci * TOKC:(tci + 1) * TOKC])
                xts.append(xt)
            # Q^T and K^T: 8 feature blocks of 128 (4 q + 4 k)
            for f in range(8):
                pqk = ps_acc.tile([128, TOKC], F32, tag="acc")
                for cc in range(n_cc):
                    nc.tensor.matmul(
                        pqk[:],
                        wqk_sb[:, cc * 2 * G_FEAT + f * 128:
                               cc * 2 * G_FEAT + f * 128 + 128],
                        xts[cc // half][:, (cc % half) * TOKC:
                                        (cc % half + 1) * TOKC],
                        start=(cc == 0), stop=(cc == n_cc - 1))
                dst = qt_sb if f < 4 else kt_sb
                fb = f % 4
                nc.vector.tensor_scalar_add(
                    dst[:, fb * T + tci * TOKC: fb * T + (tci + 1) * TOKC],
                    pqk[:], bqk_sb[:, f:f + 1])
            # V blocks (tokens on partitions), strided into VW layout
            for tb in range(TOKC // 128):
                tbg = tci * (TOKC // 128) + tb
                pv = ps_acc.tile([128, G_FEAT], F32, tag="acc")
                for cc in range(n_cc):
                    nc.tensor.matmul(
                        pv[:],
                        xts[cc // half][:, (cc % half) * TOKC + tb * 128:
                                        (cc % half) * TOKC + tb * 128 + 128],
                        wv_sb[:, cc * G_FEAT:(cc + 1) * G_FEAT],
                        start=(cc == 0), stop=False)
                nc.tensor.matmul(pv[:], ones_row[:], bv_sb[:],
                                 start=False, stop=True)
                nc.vector.tensor_copy(
                    v_sb[:, tbg * G_HEADS * VW:(tbg + 1) * G_HEADS * VW]
                    .rearrange("p (h w) -> p h w", w=VW)[:, :, 0:HD],
                    pv[:].rearrange("p (h w) -> p h w", w=HD))

        # ============ phase 2: attention, phase 3: projection ============
        for qc in range(n_qc):
            yt = ytpool.tile([128, n_hp * QC], R32, tag="yT")
            nkb = 4 * qc + 4
            for hp in range(n_hp):
                q_lo = qt_sb[0:64, hp * T + qc * QC: hp * T + (qc + 1) * QC]
                q_hi = qt_sb[64:128, hp * T + qc * QC: hp * T + (qc + 1) * QC]
                ua = ps_u.tile([HD + 1, QC], F32, tag="u")
                ub = ps_u.tile([HD + 1, QC], F32, tag="u")
                for ki in range(nkb):
                    s = ps_s.tile([128, 2 * QC], F32, tag="s")
                    nc.tensor.matmul(
                        s[:, 0:QC],
                        kt_sb[0:64, hp * T + ki * 128: hp * T + ki * 128 + 128],
                        q_lo, start=True, stop=True, tile_position=(0, 0))
                    nc.tensor.matmul(
                        s[:, QC:2 * QC],
                        kt_sb[64:128, hp * T + ki * 128: hp * T + ki * 128 + 128],
                        q_hi, start=True, stop=True, tile_position=(64, 0))
                    j = ki - 4 * qc
                    if j >= 0:  # diagonal block: causal mask
                        for hb in (0, QC):
                            if j > 0:
                                nc.vector.memset(s[:, hb:hb + 128 * j], NEG)
                            nc.vector.tensor_add(
                                s[:, hb + 128 * j: hb + 128 * (j + 1)],
                                s[:, hb + 128 * j: hb + 128 * (j + 1)],
                                mask_tri[:])
                    pt = ptpool.tile([128, 2 * QC], R32, tag="pt")
                    nc.scalar.activation(pt[:], s[:], Exp,
                                         bias=0.0, scale=0.125)
                    va = v_sb[:, (ki * G_HEADS + 2 * hp) * VW:
                              (ki * G_HEADS + 2 * hp) * VW + VW]
                    vb = v_sb[:, (ki * G_HEADS + 2 * hp + 1) * VW:
                              (ki * G_HEADS + 2 * hp + 1) * VW + VW]
                    last = (ki == nkb - 1)
                    nc.tensor.matmul(ua[:], va, pt[:, 0:QC],
                                     start=(ki == 0), stop=last)
                    nc.tensor.matmul(ub[:], vb, pt[:, QC:2 * QC],
                                     start=(ki == 0), stop=last)
                recip_a = rcpool.tile([1, QC], R32, tag="recip")
                recip_b = rcpool.tile([1, QC], R32, tag="recip")
                with nc.allow_low_precision(
                        reason="fp32r recip feeds fp32r broadcast matmul"):
                    nc.vector.reciprocal(recip_a[:], ua[HD:HD + 1, :])
                    nc.vector.reciprocal(recip_b[:], ub[HD:HD + 1, :])
                r = ps_s.tile([128, 2 * QC], F32, tag="s")
                nc.tensor.matmul(r[0:64, 0:QC], ones_row[:, 0:64],
                                 recip_a[:], start=True, stop=True)
                nc.tensor.matmul(r[0:64, QC:2 * QC], ones_row[:, 0:64],
                                 recip_b[:], start=True, stop=True)
                rb = ptpool.tile([64, 2 * QC], F32, tag="pt")
                nc.vector.tensor_copy(rb[:], r[0:64, :])
                nc.vector.tensor_mul(
                    yt[0:64, hp * QC:(hp + 1) * QC],
                    ua[0:HD, :], rb[:, 0:QC])
                ybs = ptpool.tile([64, QC], R32, tag="pt")
                nc.vector.tensor_mul(ybs[:], ub[0:HD, :], rb[:, QC:2 * QC])
                nc.sync.dma_start(
                    yt[64:128, hp * QC:(hp + 1) * QC], ybs[:])
            # projection for this query chunk
            for n in range(C // 512):
                for tb in range(QC // 128):
                    po = ps_acc.tile([128, 512], F32, tag="acc")
                    for hp in range(n_hp):
                        nc.tensor.matmul(
                            po[:],
                            yt[:, hp * QC + tb * 128: hp * QC + tb * 128 + 128],
                            wp_sb[:, hp * C + n * 512: hp * C + n * 512 + 512],
                            start=(hp == 0), stop=(hp == n_hp - 1))
                    ot = xpool.tile([128, 512], F32, tag="xT")
                    nc.vector.tensor_copy(ot[:], po[:])
                    nc.sync.dma_start(
                        part_d[qc * QC + tb * 128: qc * QC + tb * 128 + 128,
                               n * 512:(n + 1) * 512],
                        ot[:])

    nc.compile()
    return nc


def _get_program():
    if "nc" not in _CACHE:
        _CACHE["nc"] = _build_program()
    return _CACHE["nc"]


def make_in_maps(x, Wqkv, bqkv, Wproj):
    """Shard full inputs into the 8 per-core input maps."""
    x = np.asarray(x, dtype=np.float32)
    Wqkv = np.asarray(Wqkv, dtype=np.float32)
    bqkv = np.asarray(bqkv, dtype=np.float32)
    Wproj = np.asarray(Wproj, dtype=np.float32)

    xT = [np.ascontiguousarray(x[b].T) for b in range(B)]
    wqk, wv, bqk, bv, wp = [], [], [], [], []
    for g in range(2):
        qs, ks, vs = 512 * g, C + 512 * g, 2 * C + 512 * g
        wqk.append(np.ascontiguousarray(
            np.concatenate([Wqkv[:, qs:qs + 512], Wqkv[:, ks:ks + 512]], axis=1)))
        wv.append(np.ascontiguousarray(Wqkv[:, vs:vs + 512]))
        bqk.append(np.ascontiguousarray(
            np.concatenate([bqkv[qs:qs + 512], bqkv[ks:ks + 512]])))
        bv.append(np.ascontiguousarray(bqkv[vs:vs + 512].reshape(1, -1)))
        wp.append(np.ascontiguousarray(Wproj[512 * g:512 * g + 512, :]))

    maps = []
    for c in range(N_CORES):
        b, g = c // 2, c % 2
        maps.append({"xT": xT[b], "wqk": wqk[g], "wv": wv[g],
                     "bqk": bqk[g], "bv": bv[g], "wp": wp[g]})
    return maps


def kernel(x, Wqkv, bqkv, Wproj, bproj):
    from concourse.bass_utils import run_bass_kernel_spmd

    nc = _get_program()
    in_maps = make_in_maps(x, Wqkv, bqkv, Wproj)
    res = run_bass_kernel_spmd(nc, in_maps, list(range(N_CORES)))
    bproj = np.asarray(bproj, dtype=np.float32)
    out = np.empty((B, T, C), dtype=np.float32)
    for b in range(B):
        out[b] = res.results[2 * b]["part"] + res.results[2 * b + 1]["part"] + bproj
    return out



# revision 29
# speedup vs baseline: 159.3323x; 159.3323x over previous
"""Causal self-attention on 8 NeuronCores (Trainium2, Bass/Tile).

Problem: B=4, T=2048, C=1024, H=16 heads, HD=64, fp32.
    qkv = x @ Wqkv + bqkv ; causal softmax attention ; y @ Wproj + bproj

Sharding (Megatron-style): 8 cores = 4 batches x 2 head-groups.
Core c handles batch b = c//2 and head group g = c%2 (8 heads each).
Each core computes a partial output projection over its 512 head-dims;
the host sums the two partials per batch and adds bproj (the unshard step).

Per-core kernel:
  phase 1 (fp32r matmuls): qkv^T projection. Q^T,K^T produced feature-major
           [feat, tok] in bf16; V produced token-major [tok, feat] bf16 with
           a ones column per head (65-stride) so the A.V matmul also emits
           the softmax denominator; V bias dropped on-device (softmax weights
           sum to 1, so the host adds the constant bv @ Wproj instead);
           Q/K bias via DVE add (cast to bf16 on write). wqk is DMA'd in
           host-prearranged f-major blocks ordered by first use.
  phase 2: per (query chunk qc of 512, head pair hp): S^T[k,q] tiles from
           row-packed K=64 bf16 matmul pairs (2 heads share the PE array via
           tile_position row groups); ONE exp per ki on ACT (scale=1/8
           folded; subrange 2D-AP on diagonal blocks skips fully-masked
           columns); causal triangle zeroed on pt (SBUF, bf16) by a gpsimd
           affine_select - no DVE work on the critical path; A.V consumes
           P^T directly (lhsT = bf16 V block with ones col -> [65,q] psum:
           rows 0-63 unnormalized y^T, row 64 sumexp). Diagonal blocks use
           subrange matmuls (skip masked columns). Softmax division:
           recip(sumexp) on DVE, broadcast across partitions by gpsimd
           partition_broadcast, one DVE mul per head writes yt (head B's
           mul writes partitions 64-127 directly).
  phase 3: output projection partial (lhsT = bf16 y^T chunk, rhs = bf16
           Wproj rows of this head group) accumulated over 4 cin chunks.

ki order per (qc, hp): non-diagonal ascending (oldest K/V first, so
attn(qc) starts the moment attn(qc-1) drains), then diagonal blocks
descending; the start=True matmul is forced full-width so the PSUM
pending-zero region stays uniform, and stop lands on the full-width j=0
block. Program order interleaves phase1_head(i) [next chunk's hp0 q/k]
-> attn(i-1) -> phase1_tail(i) so attention owns PE priority while
phase-1 GEMMs fill its bubbles. No max-subtraction in softmax (scores
~N(0,1)), no transposes, no collectives.
"""
import sys

for _p in ("/opt/trn_rl_repo",):
    if _p not in sys.path:
        sys.path.append(_p)

import numpy as np

B, T, C = 4, 2048, 1024
H, HD = 16, 64
N_CORES = 8
G_HEADS = 8            # heads per core (one group)
G_FEAT = G_HEADS * HD  # 512 feature dims per group
VW = HD + 1            # V block stride per head (64 values + ones col)

TOKC = 512             # phase-1 token chunk
QC = 512               # phase-2 query chunk

_CACHE = {}


def _build_program():
    import contextlib
    import concourse.tile as tile
    from concourse import bacc, mybir

    F32 = mybir.dt.float32
    R32 = mybir.dt.float32r
    BF16 = mybir.dt.bfloat16
    Exp = mybir.ActivationFunctionType.Exp

    nc = bacc.Bacc("TRN2", target_bir_lowering=False, debug=False,
                   num_devices=N_CORES)

    xT_d = nc.dram_tensor("xT", [C, T], R32, kind="ExternalInput").ap()
    # f-major, host-prearranged into SBUF layout: 8 stacked [128, 1024]
    # feature blocks (block f row p col cc*128+m = Wqk[cc*128+p, f*128+m]),
    # each one contiguous DMA, so QK matmuls start after 1/8 of wqk arrives
    wqk_d = nc.dram_tensor("wqk", [8 * 128, 2 * G_FEAT], R32,
                           kind="ExternalInput").ap()
    wv_d = nc.dram_tensor("wv", [C, G_FEAT], R32, kind="ExternalInput").ap()
    bqk_d = nc.dram_tensor("bqk", [2 * G_FEAT], F32, kind="ExternalInput").ap()
    wp_d = nc.dram_tensor("wp", [G_FEAT, C], BF16, kind="ExternalInput").ap()
    part_d = nc.dram_tensor("part", [T, C], F32, kind="ExternalOutput").ap()

    n_tc = T // TOKC             # 4 phase-1 token chunks
    n_cc = C // 128              # 8 contraction chunks
    n_qc = T // QC               # 4 query chunks
    n_hp = G_HEADS // 2          # 4 head pairs
    n_tb = T // 128              # 16 token blocks

    with tile.TileContext(nc) as tc, contextlib.ExitStack() as ctx:
        const = ctx.enter_context(tc.tile_pool(name="const", bufs=1))
        wpool = ctx.enter_context(tc.tile_pool(name="weights", bufs=1))
        big = ctx.enter_context(tc.tile_pool(name="big", bufs=1))
        xpool = ctx.enter_context(tc.tile_pool(name="xT", bufs=4))
        opool = ctx.enter_context(tc.tile_pool(name="out", bufs=3))
        ytpool = ctx.enter_context(tc.tile_pool(name="yT", bufs=2))
        ptpool = ctx.enter_context(tc.tile_pool(name="pt", bufs=6))
        rcpool = ctx.enter_context(tc.tile_pool(name="recip", bufs=2))
        rbpool = ctx.enter_context(tc.tile_pool(name="rbcast", bufs=2))
        ps_acc = ctx.enter_context(
            tc.tile_pool(name="ps_acc", bufs=2, space="PSUM"))
        ps_u = ctx.enter_context(
            tc.tile_pool(name="ps_u", bufs=2, space="PSUM"))
        ps_s = ctx.enter_context(
            tc.tile_pool(name="ps_s", bufs=2, space="PSUM"))

        # ---- constants ----
        ones_f32 = const.tile([128, 128], F32)
        nc.vector.memset(ones_f32[:], 1.0)
        ones_row = const.tile([1, 128], R32)   # K=1 matmul lhsT rows
        nc.vector.tensor_copy(ones_row[:], ones_f32[0:1, :])
        # warm up the ACT exp table while DMAs stream in
        warm = const.tile([1, 2], F32)
        nc.scalar.activation(warm[:], ones_f32[0:1, 0:2], Exp,
                             bias=0.0, scale=1.0)

        half = n_cc // 2

        def load_xt_half(tci, hf):
            xt = xpool.tile([128, half * TOKC], R32, tag="xT")
            for cc in range(half):
                ccg = hf * half + cc
                nc.sync.dma_start(
                    xt[:, cc * TOKC:(cc + 1) * TOKC],
                    xT_d[ccg * 128:(ccg + 1) * 128,
                         tci * TOKC:(tci + 1) * TOKC])
            return xt

        def load_xt(tci):
            return [load_xt_half(tci, 0), load_xt_half(tci, 1)]

        # ---- resident weights; order by first use ----
        bqk_sb = wpool.tile([128, 8], F32)
        nc.sync.dma_start(bqk_sb[:], bqk_d.rearrange("(f p) -> p f", p=128))
        # f-major: wqk_sb[:, f*1024 + cc*128 : +128] = Wqk[cc-chunk, f-block]
        # DMA order matches phase-1 consumption: xt0-half, f0, xt0-half, f4,
        # f1, f5, wv, rest - the first qk_block starts at ~5 us
        wqk_sb = wpool.tile([128, n_cc * 2 * G_FEAT], R32)

        def load_f(f):
            nc.sync.dma_start(
                wqk_sb[:, f * C:(f + 1) * C],
                wqk_d[f * 128:(f + 1) * 128, :])

        xts0 = []
        xts0.append(load_xt_half(0, 0))
        load_f(0)
        xts0.append(load_xt_half(0, 1))
        load_f(4)
        for f in (1, 5):
            load_f(f)
        wv_sb = wpool.tile([128, n_cc * G_FEAT], R32)
        for cc in range(n_cc):
            nc.sync.dma_start(
                wv_sb[:, cc * G_FEAT:(cc + 1) * G_FEAT],
                wv_d[cc * 128:(cc + 1) * 128, :])
        for f in (2, 6, 3, 7):
            load_f(f)
        # wp is consumed last (phase 3) - DMA it after everything else
        wp_sb = wpool.tile([128, 4 * C], BF16)
        for cc in range(4):
            nc.sync.dma_start(
                wp_sb[:, cc * C:(cc + 1) * C],
                wp_d[cc * 128:(cc + 1) * 128, :])

        # ---- big activations ----
        qt_sb = big.tile([128, n_hp * T], BF16)  # [feat, tok] head-pair major
        kt_sb = big.tile([128, n_hp * T], BF16)
        # V: [tok-block, head, 64 vals + ones col]
        v_sb = big.tile([128, n_tb * G_HEADS * VW], BF16)
        nc.vector.tensor_copy(
            v_sb[:].rearrange("p (t w) -> p t w", w=VW)[:, :, HD:HD + 1],
            ones_f32[:].rearrange("p (a b) -> p a b", b=1))

        # ================= phase 1: qkv projection =================
        def qk_block(tci, xts, f):
            # one Q^T or K^T feature block (f<4: qt of hp f; f>=4: kt of hp f-4)
            pqk = ps_acc.tile([128, TOKC], F32, tag="acc")
            for cc in range(n_cc):
                nc.tensor.matmul(
                    pqk[:],
                    wqk_sb[:, f * C + cc * 128: f * C + cc * 128 + 128],
                    xts[cc // half][:, (cc % half) * TOKC:
                                    (cc % half + 1) * TOKC],
                    start=(cc == 0), stop=(cc == n_cc - 1))
            dst = qt_sb if f < 4 else kt_sb
            fb = f % 4
            nc.vector.tensor_scalar_add(
                dst[:, fb * T + tci * TOKC: fb * T + (tci + 1) * TOKC],
                pqk[:], bqk_sb[:, f:f + 1])

        def phase1_head(tci):
            # hp0's q+k only, emitted with high (early-program) priority so
            # attn(tci) can start the moment attn(tci-1) drains
            xts = xts0 if tci == 0 else load_xt(tci)
            qk_block(tci, xts, 0)
            qk_block(tci, xts, 4)
            return xts

        def phase1_tail(tci, xts):
            qk_block(tci, xts, 1)
            qk_block(tci, xts, 5)
            for tb in range(TOKC // 128):
                tbg = tci * (TOKC // 128) + tb
                pv = ps_acc.tile([128, G_FEAT], F32, tag="acc")
                for cc in range(n_cc):
                    nc.tensor.matmul(
                        pv[:],
                        xts[cc // half][:, (cc % half) * TOKC + tb * 128:
                                        (cc % half) * TOKC + tb * 128 + 128],
                        wv_sb[:, cc * G_FEAT:(cc + 1) * G_FEAT],
                        start=(cc == 0), stop=(cc == n_cc - 1))
                nc.vector.tensor_copy(
                    v_sb[:, tbg * G_HEADS * VW:(tbg + 1) * G_HEADS * VW]
                    .rearrange("p (h w) -> p h w", w=VW)[:, :, 0:HD],
                    pv[:].rearrange("p (h w) -> p h w", w=HD))
            for f in (2, 6, 3, 7):
                qk_block(tci, xts, f)

        # ============ phase 2: attention, phase 3: projection ============
        def attn(qc):
            yt = ytpool.tile([128, n_hp * QC], BF16, tag="yT")
            nkb = 4 * qc + 4
            # non-diagonal blocks (oldest K/V - available early) first so
            # attn(qc) overlaps phase1(qc); then diagonal blocks DESCENDING
            # so the subrange-start cascade is has_written-correct and the
            # accumulation stop lands on the full-width j=0 block
            ki_order = list(range(0, 4 * qc)) + [4 * qc + j for j in (3, 2, 1, 0)]
            for hp in range(n_hp):
                q_lo = qt_sb[0:64, hp * T + qc * QC: hp * T + (qc + 1) * QC]
                q_hi = qt_sb[64:128, hp * T + qc * QC: hp * T + (qc + 1) * QC]
                ua = ps_u.tile([HD + 1, QC], F32, tag="u")
                ub = ps_u.tile([HD + 1, QC], F32, tag="u")
                for n_ki, ki in enumerate(ki_order):
                    j = ki - 4 * qc
                    first = n_ki == 0
                    last = n_ki == nkb - 1
                    # the start=True matmul must write the full q range so
                    # the PSUM pending-zero region stays uniform (qc=0 only)
                    lo = 128 * j if (j > 0 and not first) else 0
                    s = ps_s.tile([128, 2 * QC], F32, tag="s")
                    nc.tensor.matmul(
                        s[:, lo:QC],
                        kt_sb[0:64, hp * T + ki * 128: hp * T + ki * 128 + 128],
                        q_lo[:, lo:QC], start=True, stop=True,
                        tile_position=(0, 0))
                    nc.tensor.matmul(
                        s[:, QC + lo:2 * QC],
                        kt_sb[64:128, hp * T + ki * 128: hp * T + ki * 128 + 128],
                        q_hi[:, lo:QC], start=True, stop=True,
                        tile_position=(64, 0))
                    pt = ptpool.tile([128, 2 * QC], BF16, tag="pt")
                    if lo:
                        sv = s[:].rearrange("p (h q) -> p h q", h=2)[:, :, lo:QC]
                        ptv = pt[:].rearrange("p (h q) -> p h q", h=2)[:, :, lo:QC]
                        nc.scalar.activation(ptv, sv, Exp, bias=0.0, scale=0.125)
                    else:
                        nc.scalar.activation(pt[:], s[:], Exp,
                                             bias=0.0, scale=0.125)
                    if j >= 0:
                        # zero above-diagonal entries: keep where
                        # q_local - k_local - band_offset >= 0
                        if first and j > 0:
                            # full-width variant for the start matmul:
                            # also zeroes all columns left of the band
                            ptd = pt[:].rearrange("p (h q) -> p h q", h=2)
                            base, width = -128 * j, QC
                        else:
                            ptd = pt[:].rearrange("p (h q) -> p h q",
                                                  h=2)[:, :, lo:lo + 128]
                            base, width = 0, 128
                        nc.gpsimd.affine_select(
                            out=ptd, in_=ptd,
                            compare_op=mybir.AluOpType.is_ge, fill=0.0,
                            base=base, pattern=[[0, 2], [1, width]],
                            channel_multiplier=-1)
                    va = v_sb[:, (ki * G_HEADS + 2 * hp) * VW:
                              (ki * G_HEADS + 2 * hp) * VW + VW]
                    vb = v_sb[:, (ki * G_HEADS + 2 * hp + 1) * VW:
                              (ki * G_HEADS + 2 * hp + 1) * VW + VW]
                    nc.tensor.matmul(ua[:, lo:QC], va, pt[:, lo:QC],
                                     start=first, stop=last)
                    nc.tensor.matmul(ub[:, lo:QC], vb, pt[:, QC + lo:2 * QC],
                                     start=first, stop=last)
                rc = rcpool.tile([1, 2 * QC], R32, tag="recip")
                with nc.allow_low_precision(
                        reason="fp32r recip feeds broadcast + bf16 mul"):
                    nc.vector.reciprocal(rc[:, 0:QC], ua[HD:HD + 1, :])
                    nc.vector.reciprocal(rc[:, QC:2 * QC], ub[HD:HD + 1, :])
                rb = rbpool.tile([64, 2 * QC], R32, tag="rbcast")
                nc.gpsimd.partition_broadcast(rb[:], rc[:], channels=64)
                nc.vector.tensor_mul(
                    yt[0:64, hp * QC:(hp + 1) * QC],
                    ua[0:HD, :], rb[:, 0:QC])
                nc.vector.tensor_mul(
                    yt[64:128, hp * QC:(hp + 1) * QC],
                    ub[0:HD, :], rb[:, QC:2 * QC])
            # projection for this query chunk
            for n in range(C // 512):
                for tb in range(QC // 128):
                    # ps_u (not ps_acc) so phase1(qc+1) never queues behind
                    # proj(qc) in the ps_acc pool's program-order slots
                    po = ps_u.tile([128, 512], F32, tag="u")
                    for hp in range(n_hp):
                        nc.tensor.matmul(
                            po[:],
                            yt[:, hp * QC + tb * 128: hp * QC + tb * 128 + 128],
                            wp_sb[:, hp * C + n * 512: hp * C + n * 512 + 512],
                            start=(hp == 0), stop=(hp == n_hp - 1))
                    ot = opool.tile([128, 512], F32, tag="out")
                    nc.vector.tensor_copy(ot[:], po[:])
                    nc.sync.dma_start(
                        part_d[qc * QC + tb * 128: qc * QC + tb * 128 + 128,
                               n * 512:(n + 1) * 512],
                        ot[:])

        # pipeline: head(i) [next chunk's hp0 q/k] gets priority before
        # attn(i-1); tail(i) [V + remaining q/k] fills attn(i-1)'s PE
        # bubbles; attn(i) then starts the moment attn(i-1) drains.
        xts_c = phase1_head(0)
        phase1_tail(0, xts_c)
        for i in range(1, n_tc):
            xts_c = phase1_head(i)
            attn(i - 1)
            phase1_tail(i, xts_c)
        attn(n_qc - 1)

    nc.compile()
    return nc


def _get_program():
    if "nc" not in _CACHE:
        _CACHE["nc"] = _build_program()
    return _CACHE["nc"]


def make_in_maps(x, Wqkv, bqkv, Wproj):
    """Shard full inputs into the 8 per-core input maps."""
    import ml_dtypes

    x = np.asarray(x, dtype=np.float32)
    Wqkv = np.asarray(Wqkv, dtype=np.float32)
    bqkv = np.asarray(bqkv, dtype=np.float32)
    Wproj = np.asarray(Wproj, dtype=np.float32)

    xT = [np.ascontiguousarray(x[b].T) for b in range(B)]
    wqk, wv, bqk, bv, wp = [], [], [], [], []
    for g in range(2):
        qs, ks, vs = 512 * g, C + 512 * g, 2 * C + 512 * g
        # f-major SBUF prearrangement: [f, p, cc, m] stacking of the
        # [C, 1024] q|k weight block (see wqk_d comment in _build_program)
        w = np.concatenate([Wqkv[:, qs:qs + 512], Wqkv[:, ks:ks + 512]],
                           axis=1)
        w = w.reshape(8, 128, 8, 128).transpose(2, 1, 0, 3)
        wqk.append(np.ascontiguousarray(w.reshape(8 * 128, 8 * 128)))
        wv.append(np.ascontiguousarray(Wqkv[:, vs:vs + 512]))
        bqk.append(np.ascontiguousarray(
            np.concatenate([bqkv[qs:qs + 512], bqkv[ks:ks + 512]])))
        wp.append(np.ascontiguousarray(
            Wproj[512 * g:512 * g + 512, :].astype(ml_dtypes.bfloat16)))

    maps = []
    for c in range(N_CORES):
        b, g = c // 2, c % 2
        maps.append({"xT": xT[b], "wqk": wqk[g], "wv": wv[g],
                     "bqk": bqk[g], "wp": wp[g]})
    return maps


def kernel(x, Wqkv, bqkv, Wproj, bproj):
    from concourse.bass_utils import run_bass_kernel_spmd

    nc = _get_program()
    in_maps = make_in_maps(x, Wqkv, bqkv, Wproj)
    res = run_bass_kernel_spmd(nc, in_maps, list(range(N_CORES)))
    bproj = np.asarray(bproj, dtype=np.float32)
    bqkv = np.asarray(bqkv, dtype=np.float32)
    Wproj = np.asarray(Wproj, dtype=np.float32)
    # V-bias correction: softmax weights are row-stochastic, so dropping bv
    # on-device shifts y by exactly -bv; out is short by the constant bv @ Wproj.
    bias = bproj + bqkv[2 * C:3 * C] @ Wproj
    out = np.empty((B, T, C), dtype=np.float32)
    for b in range(B):
        out[b] = res.results[2 * b]["part"] + res.results[2 * b + 1]["part"] + bias
    return out


# revision 33
# speedup vs baseline: 160.8279x; 1.0094x over previous
"""Causal self-attention on 8 NeuronCores (Trainium2, Bass/Tile).

Problem: B=4, T=2048, C=1024, H=16 heads, HD=64, fp32.
    qkv = x @ Wqkv + bqkv ; causal softmax attention ; y @ Wproj + bproj

Sharding (Megatron-style): 8 cores = 4 batches x 2 head-groups.
Core c handles batch b = c//2 and head group g = c%2 (8 heads each).
Each core computes a partial output projection over its 512 head-dims;
the host sums the two partials per batch and adds bproj (the unshard step).

Per-core kernel:
  phase 1 (fp32r matmuls): qkv^T projection. Q^T,K^T produced feature-major
           [feat, tok] in bf16; V produced token-major [tok, feat] bf16 with
           a ones column per head (65-stride) so the A.V matmul also emits
           the softmax denominator; V bias dropped on-device (softmax weights
           sum to 1, so the host adds the constant bv @ Wproj instead);
           Q/K bias via DVE add (cast to bf16 on write). wqk is DMA'd in
           host-prearranged f-major blocks ordered by first use.
  phase 2: per (query chunk qc of 512, head pair hp): S^T[k,q] tiles from
           row-packed K=64 bf16 matmul pairs (2 heads share the PE array via
           tile_position row groups); ONE exp per ki on ACT (scale=1/8
           folded; subrange 2D-AP on diagonal blocks skips fully-masked
           columns); causal triangle zeroed on pt (SBUF, bf16) by a gpsimd
           affine_select - no DVE work on the critical path; A.V consumes
           P^T directly (lhsT = bf16 V block with ones col -> [65,q] psum:
           rows 0-63 unnormalized y^T, row 64 sumexp). Diagonal blocks use
           subrange matmuls (skip masked columns). Softmax division:
           recip(sumexp) on DVE, broadcast across partitions by gpsimd
           partition_broadcast, one DVE mul per head writes yt (head B's
           mul writes partitions 64-127 directly).
  phase 3: output projection partial (lhsT = bf16 y^T chunk, rhs = bf16
           Wproj rows of this head group) accumulated over 4 cin chunks.

ki order per (qc, hp): non-diagonal ascending (oldest K/V first, so
attn(qc) starts the moment attn(qc-1) drains), then diagonal blocks
descending; the start=True matmul is forced full-width so the PSUM
pending-zero region stays uniform, and stop lands on the full-width j=0
block. Program order interleaves phase1_head(i) [next chunk's hp0 q/k]
-> attn(i-1) -> phase1_tail(i) so attention owns PE priority while
phase-1 GEMMs fill its bubbles. No max-subtraction in softmax (scores
~N(0,1)), no transposes, no collectives.
"""
import sys

for _p in ("/opt/trn_rl_repo",):
    if _p not in sys.path:
        sys.path.append(_p)

import numpy as np

B, T, C = 4, 2048, 1024
H, HD = 16, 64
N_CORES = 8
G_HEADS = 8            # heads per core (one group)
G_FEAT = G_HEADS * HD  # 512 feature dims per group
VW = HD + 1            # V block stride per head (64 values + ones col)

TOKC = 512             # phase-1 token chunk
QC = 512               # phase-2 query chunk

_CACHE = {}


def _build_program():
    import contextlib
    import concourse.tile as tile
    from concourse import bacc, mybir

    F32 = mybir.dt.float32
    R32 = mybir.dt.float32r
    BF16 = mybir.dt.bfloat16
    Exp = mybir.ActivationFunctionType.Exp

    nc = bacc.Bacc("TRN2", target_bir_lowering=False, debug=False,
                   num_devices=N_CORES)

    xT_d = nc.dram_tensor("xT", [C, T], R32, kind="ExternalInput").ap()
    # f-major, host-prearranged into SBUF layout: 8 stacked [128, 1024]
    # feature blocks (block f row p col cc*128+m = Wqk[cc*128+p, f*128+m]),
    # each one contiguous DMA, so QK matmuls start after 1/8 of wqk arrives
    wqk_d = nc.dram_tensor("wqk", [8 * 128, 2 * G_FEAT], R32,
                           kind="ExternalInput").ap()
    wv_d = nc.dram_tensor("wv", [C, G_FEAT], R32, kind="ExternalInput").ap()
    bqk_d = nc.dram_tensor("bqk", [2 * G_FEAT], F32, kind="ExternalInput").ap()
    wp_d = nc.dram_tensor("wp", [G_FEAT, C], BF16, kind="ExternalInput").ap()
    part_d = nc.dram_tensor("part", [T, C], F32, kind="ExternalOutput").ap()

    n_tc = T // TOKC             # 4 phase-1 token chunks
    n_cc = C // 128              # 8 contraction chunks
    n_qc = T // QC               # 4 query chunks
    n_hp = G_HEADS // 2          # 4 head pairs
    n_tb = T // 128              # 16 token blocks

    with tile.TileContext(nc) as tc, contextlib.ExitStack() as ctx:
        const = ctx.enter_context(tc.tile_pool(name="const", bufs=1))
        wpool = ctx.enter_context(tc.tile_pool(name="weights", bufs=1))
        big = ctx.enter_context(tc.tile_pool(name="big", bufs=1))
        xpool = ctx.enter_context(tc.tile_pool(name="xT", bufs=4))
        opool = ctx.enter_context(tc.tile_pool(name="out", bufs=3))
        ytpool = ctx.enter_context(tc.tile_pool(name="yT", bufs=2))
        ptpool = ctx.enter_context(tc.tile_pool(name="pt", bufs=6))
        rcpool = ctx.enter_context(tc.tile_pool(name="recip", bufs=2))
        rbpool = ctx.enter_context(tc.tile_pool(name="rbcast", bufs=2))
        ps_acc = ctx.enter_context(
            tc.tile_pool(name="ps_acc", bufs=2, space="PSUM"))
        ps_u = ctx.enter_context(
            tc.tile_pool(name="ps_u", bufs=2, space="PSUM"))
        ps_s = ctx.enter_context(
            tc.tile_pool(name="ps_s", bufs=2, space="PSUM"))

        # ---- constants ----
        ones_f32 = const.tile([128, 128], F32)
        nc.vector.memset(ones_f32[:], 1.0)
        ones_row = const.tile([1, 128], R32)   # K=1 matmul lhsT rows
        nc.vector.tensor_copy(ones_row[:], ones_f32[0:1, :])
        # warm up the ACT exp table while DMAs stream in
        warm = const.tile([1, 2], F32)
        nc.scalar.activation(warm[:], ones_f32[0:1, 0:2], Exp,
                             bias=0.0, scale=1.0)

        half = n_cc // 2

        def load_xt_half(tci, hf):
            xt = xpool.tile([128, half * TOKC], R32, tag="xT")
            for cc in range(half):
                ccg = hf * half + cc
                nc.sync.dma_start(
                    xt[:, cc * TOKC:(cc + 1) * TOKC],
                    xT_d[ccg * 128:(ccg + 1) * 128,
                         tci * TOKC:(tci + 1) * TOKC])
            return xt

        def load_xt(tci):
            return [load_xt_half(tci, 0), load_xt_half(tci, 1)]

        # ---- resident weights; order by first use ----
        bqk_sb = wpool.tile([128, 8], F32)
        nc.sync.dma_start(bqk_sb[:], bqk_d.rearrange("(f p) -> p f", p=128))
        # f-major: wqk_sb[:, f*1024 + cc*128 : +128] = Wqk[cc-chunk, f-block]
        # DMA order matches phase-1 consumption: xt0-half, f0, xt0-half, f4,
        # f1, f5, wv, rest - the first qk_block starts at ~5 us
        wqk_sb = wpool.tile([128, n_cc * 2 * G_FEAT], R32)

        def load_f(f):
            nc.sync.dma_start(
                wqk_sb[:, f * C:(f + 1) * C],
                wqk_d[f * 128:(f + 1) * 128, :])

        xts0 = []
        xts0.append(load_xt_half(0, 0))
        load_f(0)
        xts0.append(load_xt_half(0, 1))
        load_f(4)
        for f in (1, 5):
            load_f(f)
        wv_sb = wpool.tile([128, n_cc * G_FEAT], R32)
        for cc in range(n_cc):
            nc.sync.dma_start(
                wv_sb[:, cc * G_FEAT:(cc + 1) * G_FEAT],
                wv_d[cc * 128:(cc + 1) * 128, :])
        for f in (2, 6, 3, 7):
            load_f(f)
        # wp is consumed last (phase 3) - DMA it after everything else
        wp_sb = wpool.tile([128, 4 * C], BF16)
        for cc in range(4):
            nc.sync.dma_start(
                wp_sb[:, cc * C:(cc + 1) * C],
                wp_d[cc * 128:(cc + 1) * 128, :])

        # ---- big activations ----
        qt_sb = big.tile([128, n_hp * T], BF16)  # [feat, tok] head-pair major
        kt_sb = big.tile([128, n_hp * T], BF16)
        # V: [tok-block, head, 64 vals + ones col]
        v_sb = big.tile([128, n_tb * G_HEADS * VW], BF16)
        nc.vector.tensor_copy(
            v_sb[:].rearrange("p (t w) -> p t w", w=VW)[:, :, HD:HD + 1],
            ones_f32[:].rearrange("p (a b) -> p a b", b=1))

        # ================= phase 1: qkv projection =================
        def qk_block(tci, xts, f):
            # one Q^T or K^T feature block (f<4: qt of hp f; f>=4: kt of hp f-4)
            pqk = ps_acc.tile([128, TOKC], F32, tag="acc")
            for cc in range(n_cc):
                nc.tensor.matmul(
                    pqk[:],
                    wqk_sb[:, f * C + cc * 128: f * C + cc * 128 + 128],
                    xts[cc // half][:, (cc % half) * TOKC:
                                    (cc % half + 1) * TOKC],
                    start=(cc == 0), stop=(cc == n_cc - 1))
            dst = qt_sb if f < 4 else kt_sb
            fb = f % 4
            nc.vector.tensor_scalar_add(
                dst[:, fb * T + tci * TOKC: fb * T + (tci + 1) * TOKC],
                pqk[:], bqk_sb[:, f:f + 1])

        def phase1_head(tci):
            # hp0's q+k only, emitted with high (early-program) priority so
            # attn(tci) can start the moment attn(tci-1) drains
            xts = xts0 if tci == 0 else load_xt(tci)
            qk_block(tci, xts, 0)
            qk_block(tci, xts, 4)
            return xts

        def phase1_tail(tci, xts):
            qk_block(tci, xts, 1)
            qk_block(tci, xts, 5)
            for tb in range(TOKC // 128):
                tbg = tci * (TOKC // 128) + tb
                pv = ps_acc.tile([128, G_FEAT], F32, tag="acc")
                for cc in range(n_cc):
                    nc.tensor.matmul(
                        pv[:],
                        xts[cc // half][:, (cc % half) * TOKC + tb * 128:
                                        (cc % half) * TOKC + tb * 128 + 128],
                        wv_sb[:, cc * G_FEAT:(cc + 1) * G_FEAT],
                        start=(cc == 0), stop=(cc == n_cc - 1))
                nc.vector.tensor_copy(
                    v_sb[:, tbg * G_HEADS * VW:(tbg + 1) * G_HEADS * VW]
                    .rearrange("p (h w) -> p h w", w=VW)[:, :, 0:HD],
                    pv[:].rearrange("p (h w) -> p h w", w=HD))
            for f in (2, 6, 3, 7):
                qk_block(tci, xts, f)

        # ============ phase 2: attention, phase 3: projection ============
        def attn(qc):
            yt = ytpool.tile([128, n_hp * QC], BF16, tag="yT")
            # non-diagonal blocks (oldest K/V - available early) first so
            # attn(qc) overlaps phase1(qc); then diagonal blocks DESCENDING
            # so the subrange-start cascade is has_written-correct and the
            # accumulation stop lands on the full-width j=0 block
            units = [(k,) for k in range(0, 4 * qc)]
            units += [(4 * qc + j,) for j in (3, 2, 1, 0)]
            for hp in range(n_hp):
                q_lo = qt_sb[0:64, hp * T + qc * QC: hp * T + (qc + 1) * QC]
                q_hi = qt_sb[64:128, hp * T + qc * QC: hp * T + (qc + 1) * QC]
                ua = ps_u.tile([HD + 1, QC], F32, tag="u")
                ub = ps_u.tile([HD + 1, QC], F32, tag="u")

                def vav(ki, h):
                    o = (ki * G_HEADS + 2 * hp + h) * VW
                    return v_sb[:, o:o + VW]

                for iu, unit in enumerate(units):
                    first = iu == 0
                    last = iu == len(units) - 1
                    ki = unit[0]
                    j = ki - 4 * qc
                    # the start=True matmul must write the full q range so
                    # the PSUM pending-zero region stays uniform (qc=0 only)
                    lo = 128 * j if (j > 0 and not first) else 0
                    s = ps_s.tile([128, 2 * QC], F32, tag="s")
                    ks = hp * T + ki * 128
                    nc.tensor.matmul(
                        s[:, lo:QC], kt_sb[0:64, ks:ks + 128],
                        q_lo[:, lo:QC], start=True, stop=True,
                        tile_position=(0, 0))
                    nc.tensor.matmul(
                        s[:, QC + lo:2 * QC], kt_sb[64:128, ks:ks + 128],
                        q_hi[:, lo:QC], start=True, stop=True,
                        tile_position=(64, 0))
                    pt = ptpool.tile([128, 2 * QC], BF16, tag="pt")
                    if lo:
                        sv = s[:].rearrange("p (h q) -> p h q", h=2)[:, :, lo:QC]
                        ptv = pt[:].rearrange("p (h q) -> p h q", h=2)[:, :, lo:QC]
                        nc.scalar.activation(ptv, sv, Exp, bias=0.0, scale=0.125)
                    else:
                        nc.scalar.activation(pt[:], s[:], Exp,
                                             bias=0.0, scale=0.125)
                    if j >= 0:
                        # zero above-diagonal entries: keep where
                        # q_local - k_local - band_offset >= 0
                        if first and j > 0:
                            # full-width variant for the start matmul:
                            # also zeroes all columns left of the band
                            ptd = pt[:].rearrange("p (h q) -> p h q", h=2)
                            base, width = -128 * j, QC
                        else:
                            ptd = pt[:].rearrange("p (h q) -> p h q",
                                                  h=2)[:, :, lo:lo + 128]
                            base, width = 0, 128
                        nc.gpsimd.affine_select(
                            out=ptd, in_=ptd,
                            compare_op=mybir.AluOpType.is_ge, fill=0.0,
                            base=base, pattern=[[0, 2], [1, width]],
                            channel_multiplier=-1)
                    nc.tensor.matmul(ua[:, lo:QC], vav(ki, 0), pt[:, lo:QC],
                                     start=first, stop=last)
                    nc.tensor.matmul(ub[:, lo:QC], vav(ki, 1),
                                     pt[:, QC + lo:2 * QC],
                                     start=first, stop=last)
                # per-head recip/broadcast/mul (separate full tiles:
                # partition_broadcast mis-lowers subrange APs) so head A's
                # mul - which frees the ua slot gating the next pair's AV -
                # skips head B's chain entirely
                rc_a = rcpool.tile([1, QC], R32, tag="recip")
                rc_b = rcpool.tile([1, QC], R32, tag="recip")
                with nc.allow_low_precision(
                        reason="fp32r recip feeds broadcast + bf16 mul"):
                    nc.vector.reciprocal(rc_a[:], ua[HD:HD + 1, :])
                    nc.vector.reciprocal(rc_b[:], ub[HD:HD + 1, :])
                rb_a = rbpool.tile([64, QC], R32, tag="rbcast")
                nc.gpsimd.partition_broadcast(rb_a[:], rc_a[:], channels=64)
                nc.vector.tensor_mul(
                    yt[0:64, hp * QC:(hp + 1) * QC], ua[0:HD, :], rb_a[:])
                rb_b = rbpool.tile([64, QC], R32, tag="rbcast")
                nc.gpsimd.partition_broadcast(rb_b[:], rc_b[:], channels=64)
                nc.vector.tensor_mul(
                    yt[64:128, hp * QC:(hp + 1) * QC], ub[0:HD, :], rb_b[:])
            # projection for this query chunk
            for n in range(C // 512):
                for tb in range(QC // 128):
                    # ps_u (not ps_acc) so phase1(qc+1) never queues behind
                    # proj(qc) in the ps_acc pool's program-order slots
                    po = ps_u.tile([128, 512], F32, tag="u")
                    for hp in range(n_hp):
                        nc.tensor.matmul(
                            po[:],
                            yt[:, hp * QC + tb * 128: hp * QC + tb * 128 + 128],
                            wp_sb[:, hp * C + n * 512: hp * C + n * 512 + 512],
                            start=(hp == 0), stop=(hp == n_hp - 1))
                    ot = opool.tile([128, 512], F32, tag="out")
                    nc.vector.tensor_copy(ot[:], po[:])
                    nc.sync.dma_start(
                        part_d[qc * QC + tb * 128: qc * QC + tb * 128 + 128,
                               n * 512:(n + 1) * 512],
                        ot[:])

        # pipeline: head(i) [next chunk's hp0 q/k] gets priority before
        # attn(i-1); tail(i) [V + remaining q/k] fills attn(i-1)'s PE
        # bubbles; attn(i) then starts the moment attn(i-1) drains.
        xts_c = phase1_head(0)
        phase1_tail(0, xts_c)
        for i in range(1, n_tc):
            xts_c = phase1_head(i)
            attn(i - 1)
            phase1_tail(i, xts_c)
        attn(n_qc - 1)

    nc.compile()
    return nc


def _get_program():
    if "nc" not in _CACHE:
        _CACHE["nc"] = _build_program()
    return _CACHE["nc"]


def make_in_maps(x, Wqkv, bqkv, Wproj):
    """Shard full inputs into the 8 per-core input maps."""
    import ml_dtypes

    x = np.asarray(x, dtype=np.float32)
    Wqkv = np.asarray(Wqkv, dtype=np.float32)
    bqkv = np.asarray(bqkv, dtype=np.float32)
    Wproj = np.asarray(Wproj, dtype=np.float32)

    xT = [np.ascontiguousarray(x[b].T) for b in range(B)]
    wqk, wv, bqk, bv, wp = [], [], [], [], []
    for g in range(2):
        qs, ks, vs = 512 * g, C + 512 * g, 2 * C + 512 * g
        # f-major SBUF prearrangement: [f, p, cc, m] stacking of the
        # [C, 1024] q|k weight block (see wqk_d comment in _build_program)
        w = np.concatenate([Wqkv[:, qs:qs + 512], Wqkv[:, ks:ks + 512]],
                           axis=1)
        w = w.reshape(8, 128, 8, 128).transpose(2, 1, 0, 3)
        wqk.append(np.ascontiguousarray(w.reshape(8 * 128, 8 * 128)))
        wv.append(np.ascontiguousarray(Wqkv[:, vs:vs + 512]))
        bqk.append(np.ascontiguousarray(
            np.concatenate([bqkv[qs:qs + 512], bqkv[ks:ks + 512]])))
        wp.append(np.ascontiguousarray(
            Wproj[512 * g:512 * g + 512, :].astype(ml_dtypes.bfloat16)))

    maps = []
    for c in range(N_CORES):
        b, g = c // 2, c % 2
        maps.append({"xT": xT[b], "wqk": wqk[g], "wv": wv[g],
                     "bqk": bqk[g], "wp": wp[g]})
    return maps


def kernel(x, Wqkv, bqkv, Wproj, bproj):
    from concourse.bass_utils import run_bass_kernel_spmd

    nc = _get_program()
    in_maps = make_in_maps(x, Wqkv, bqkv, Wproj)
    res = run_bass_kernel_spmd(nc, in_maps, list(range(N_CORES)))
    bproj = np.asarray(bproj, dtype=np.float32)
    bqkv = np.asarray(bqkv, dtype=np.float32)
    Wproj = np.asarray(Wproj, dtype=np.float32)
    # V-bias correction: softmax weights are row-stochastic, so dropping bv
    # on-device shifts y by exactly -bv; out is short by the constant bv @ Wproj.
    bias = bproj + bqkv[2 * C:3 * C] @ Wproj
    out = np.empty((B, T, C), dtype=np.float32)
    for b in range(B):
        out[b] = res.results[2 * b]["part"] + res.results[2 * b + 1]["part"] + bias
    return out
